# revision 1
# baseline (speedup 1.0000x reference)
"""CaptioningRNN forward loss on 8 Trainium2 NeuronCores.

Data-parallel over N: each core handles 16 of the 128 sequences end-to-end
(embed gather, xW precompute, sequential RNN scan, vocab scores + fused
softmax-CE reduction); the scalar loss is all-reduced at the end.

Problem shapes (hardcoded): N=128, T=33 (32 steps), Dfeat=512, W=512,
H=1024, V=16384. All matmuls run in float32r (full-rate fp32 on the PE).
"""
import numpy as np
import concourse.bass as bass
import concourse.tile as tile
from concourse import bacc, mybir
from concourse.bass_utils import run_bass_kernel_spmd
from concourse.masks import make_identity
from contextlib import ExitStack

dt = mybir.dt
AF = mybir.ActivationFunctionType
OP = mybir.AluOpType

N_CORES = 8
NL = 16          # sequences per core
T_STEPS = 32     # scan steps (T-1)
DF = 512         # feature dim
WD = 512         # word vec dim
H = 1024         # hidden dim
V = 16384        # vocab
NTOK = NL * T_STEPS          # 512 tokens per core (t-major: tok = t*16 + n)
NG = NTOK // 128             # 4 groups of 128 tokens
NJ = V // 512                # 32 vocab column tiles
KC_H = H // 128              # 8 contraction chunks over H
KC_W = WD // 128             # 4 contraction chunks over W

_nc_cache = None


def build_program():
    global _nc_cache
    if _nc_cache is not None:
        return _nc_cache
    import os
    kphases = int(os.environ.get("KPHASES", "4"))
    kloop = int(os.environ.get("KLOOP", "1"))
    kscan_nodeps = int(os.environ.get("KSCAN_NODEPS", "0"))
    kscan_bare = int(os.environ.get("KSCAN_BARE", "0"))
    kscan_same = int(os.environ.get("KSCAN_SAME", "0"))
    nc = bacc.Bacc("TRN2", target_bir_lowering=False, debug=False,
                   num_devices=N_CORES)

    # ---- DRAM parameters (per-core shards / replicated weights) ----
    feat_d = nc.dram_tensor("features", [NL, DF], dt.float32, kind="ExternalInput")
    tok_d = nc.dram_tensor("tok", [128, NG], dt.int32, kind="ExternalInput")
    yrel_d = nc.dram_tensor("yrel", [128, NG, NJ], dt.float32, kind="ExternalInput")
    maskn_d = nc.dram_tensor("maskn", [128, NG], dt.float32, kind="ExternalInput")
    wembed_d = nc.dram_tensor("W_embed", [V, WD], dt.float32, kind="ExternalInput")
    wproj_d = nc.dram_tensor("W_proj", [DF, H], dt.float32r, kind="ExternalInput")
    bproj_d = nc.dram_tensor("b_proj", [1, H], dt.float32r, kind="ExternalInput")
    wx_d = nc.dram_tensor("Wx", [WD, H], dt.float32r, kind="ExternalInput")
    wh_d = nc.dram_tensor("Wh", [H, H], dt.float32r, kind="ExternalInput")
    brnn_d = nc.dram_tensor("b_rnn", [1, H], dt.float32r, kind="ExternalInput")
    wv_d = nc.dram_tensor("WVt", [KC_H, NJ, 128, 512], dt.float32r,
                          kind="ExternalInput")
    bvoc_d = nc.dram_tensor("b_vocab", [NJ, 512], dt.float32r, kind="ExternalInput")
    loss_d = nc.dram_tensor("loss", [1, 4], dt.float32, kind="ExternalOutput")
    if kphases < 4:
        dbg_xw = nc.dram_tensor("dbg_xw", [128, NG, H], dt.float32, kind="ExternalOutput")
        dbg_ht = nc.dram_tensor("dbg_ht", [128, KC_H, 128], dt.float32, kind="ExternalOutput")
        dbg_sc = nc.dram_tensor("dbg_sc", [128, NG, NJ], dt.float32, kind="ExternalOutput")

    cc_in = nc.dram_tensor("cc_in", [128, 4], dt.float32)
    cc_out = nc.dram_tensor("cc_out", [128, 4], dt.float32, addr_space="Shared")

    kunroll = int(os.environ.get("KUNROLL", "1"))
    with tile.TileContext(nc) as tc, ExitStack() as ctx:
        if kloop > 1:
            ctx.enter_context(tc.For_i(0, kloop, 1))
        const = ctx.enter_context(tc.tile_pool(name="const", bufs=1))
        acts = ctx.enter_context(tc.tile_pool(name="acts", bufs=1))
        wts = ctx.enter_context(tc.tile_pool(name="wts", bufs=1))
        scr = ctx.enter_context(tc.tile_pool(name="scr", bufs=2))
        psA = ctx.enter_context(tc.tile_pool(name="psA", bufs=2, space="PSUM"))
        psB = ctx.enter_context(tc.tile_pool(name="psB", bufs=2, space="PSUM"))

        # ---- constants / small inputs ----
        ident128 = const.tile([128, 128], dt.float32)
        make_identity(nc, ident128[:])
        ident16 = const.tile([16, 16], dt.float32)
        make_identity(nc, ident16[:])
        ident128r = const.tile([128, 128], dt.float32r)
        nc.vector.tensor_copy(ident128r[:], ident128[:])
        iota_i = const.tile([128, 512], dt.int32)
        nc.gpsimd.iota(iota_i[:], pattern=[[1, 512]], base=0, channel_multiplier=0)
        iota_f = const.tile([128, 512], dt.float32)
        nc.vector.tensor_copy(iota_f[:], iota_i[:])
        ones16 = const.tile([1, 16], dt.float32r)
        nc.vector.memset(ones16[:].bitcast(dt.float32), 1.0)
        ones128 = const.tile([1, 128], dt.float32r)
        nc.vector.memset(ones128[:].bitcast(dt.float32), 1.0)
        onescol = const.tile([128, 1], dt.float32)
        nc.vector.memset(onescol[:], 1.0)

        tok_t = const.tile([128, NG], dt.int32)
        nc.sync.dma_start(tok_t[:], tok_d.ap())
        yrel_t = const.tile([128, NG, NJ], dt.float32)
        nc.sync.dma_start(yrel_t[:], yrel_d.ap())
        maskn_t = const.tile([128, NG], dt.float32)
        nc.sync.dma_start(maskn_t[:], maskn_d.ap())
        feat_t = const.tile([NL, DF], dt.float32)
        nc.sync.dma_start(feat_t[:], feat_d.ap())
        bproj_t = const.tile([1, H], dt.float32r)
        nc.sync.dma_start(bproj_t[:], bproj_d.ap())
        brnn_t = const.tile([1, H], dt.float32r)
        nc.sync.dma_start(brnn_t[:], brnn_d.ap())

        # ---- persistent activations ----
        hT0 = acts.tile([128, KC_H, 16], dt.float32r)        # h0 transposed
        hTm = [acts.tile([128, KC_H, 128], dt.float32r, tag=f"hTm{m}",
                         name=f"hTm{m}")
               for m in range(NG)]                            # h1..h32 transposed
        xw_all = acts.tile([128, NG, H], dt.float32r)         # x @ Wx + b
        s_cols = acts.tile([128, NG, NJ], dt.float32)         # exp-sum partials
        t_cols = acts.tile([128, NG, NJ], dt.float32)         # target partials

        # ---- phase 1: embed gather + xT + xW, h0 ----
        with ExitStack() as ectx:
            early = ectx.enter_context(tc.tile_pool(name="early", bufs=1))
            wp_t = early.tile([128, KC_W, H], dt.float32r)
            for kc in range(KC_W):
                nc.sync.dma_start(wp_t[:, kc, :], wproj_d.ap()[kc * 128:(kc + 1) * 128, :])
            wx_t = early.tile([128, KC_W, H], dt.float32r)
            for kc in range(KC_W):
                nc.sync.dma_start(wx_t[:, kc, :], wx_d.ap()[kc * 128:(kc + 1) * 128, :])
            x_all = early.tile([128, NG, WD], dt.float32)
            for g in range(NG):
                nc.gpsimd.indirect_dma_start(
                    out=x_all[:, g, :], out_offset=None,
                    in_=wembed_d.ap(),
                    in_offset=bass.IndirectOffsetOnAxis(ap=tok_t[:, g:g + 1], axis=0),
                )
            xT_all = early.tile([128, KC_W, NTOK], dt.float32r)
            for g in range(NG):
                ps_x = psB.tile([128, KC_W, 128], dt.float32, space="PSUM", tag="big")
                for wc in range(KC_W):
                    nc.tensor.transpose(out=ps_x[:, wc, :],
                                        in_=x_all[:, g, wc * 128:(wc + 1) * 128],
                                        identity=ident128[:])
                nc.vector.tensor_copy(xT_all[:, :, g * 128:(g + 1) * 128], ps_x[:])

            # xW = x @ Wx + b  (per token group, 512-wide halves)
            for g in range(NG):
                for hf in range(2):
                    ps = psB.tile([128, 512], dt.float32, space="PSUM", tag="big")
                    for wc in range(KC_W):
                        nc.tensor.matmul(
                            out=ps[:],
                            lhsT=xT_all[:, wc, g * 128:(g + 1) * 128],
                            rhs=wx_t[:, wc, hf * 512:(hf + 1) * 512],
                            start=(wc == 0), stop=False)
                    nc.tensor.matmul(out=ps[:], lhsT=ones128[:],
                                     rhs=brnn_t[:, hf * 512:(hf + 1) * 512],
                                     start=False, stop=True)
                    nc.vector.tensor_copy(xw_all[:, g, hf * 512:(hf + 1) * 512], ps[:])

            # h0 = features @ W_proj + b_proj
            featT = early.tile([128, KC_W, 16], dt.float32r)
            ps_t0 = psB.tile([128, KC_H, 16], dt.float32, space="PSUM", tag="tr")
            for kc in range(KC_W):
                nc.tensor.transpose(out=ps_t0[:, kc, :],
                                    in_=feat_t[:, kc * 128:(kc + 1) * 128],
                                    identity=ident16[:])
            nc.vector.tensor_copy(featT[:], ps_t0[:, :KC_W, :])
            ps_h = psA.tile([NL, H], dt.float32, space="PSUM", tag="scan")
            for hf in range(2):
                for kc in range(KC_W):
                    nc.tensor.matmul(
                        out=ps_h[:, hf * 512:(hf + 1) * 512],
                        lhsT=featT[:, kc, :],
                        rhs=wp_t[:, kc, hf * 512:(hf + 1) * 512],
                        start=(kc == 0), stop=False)
                nc.tensor.matmul(out=ps_h[:, hf * 512:(hf + 1) * 512],
                                 lhsT=ones16[:], rhs=bproj_t[:, hf * 512:(hf + 1) * 512],
                                 start=False, stop=True)
            h_scr0 = scr.tile([NL, H], dt.float32, tag="hscr")
            nc.scalar.copy(h_scr0[:], ps_h[:])
            ps_tr = psB.tile([128, KC_H, 16], dt.float32, space="PSUM", tag="tr")
            for kc in range(KC_H):
                nc.tensor.transpose(out=ps_tr[:, kc, :],
                                    in_=h_scr0[:, kc * 128:(kc + 1) * 128],
                                    identity=ident16[:])
            nc.vector.tensor_copy(hT0[:], ps_tr[:])

        if kphases == 1:
            nc.sync.dma_start(dbg_xw.ap(), xw_all[:].bitcast(dt.float32))
            nc.sync.dma_start(dbg_ht.ap()[:, :, :16], hT0[:].bitcast(dt.float32))

        # ---- phase 2: Wh load + scan ----
        wh_t = wts.tile([128, KC_H, H], dt.float32r)
        for kc in range(KC_H):
            nc.sync.dma_start(wh_t[:, kc, :], wh_d.ap()[kc * 128:(kc + 1) * 128, :])

        def emit_inject(ps, b):
            m, i = b // 8, b % 8
            for hf in range(2):
                nc.tensor.matmul(
                    out=ps[:, hf * 512:(hf + 1) * 512],
                    lhsT=ident128r[:, i * 16:i * 16 + 16],
                    rhs=xw_all[:, m, hf * 512:(hf + 1) * 512],
                    start=True, stop=False)

        ps_pending = None
        for t in range(1, (T_STEPS if kphases >= 2 else 0) + 1):
            b = t - 1                      # token block index [0,32)
            m, i = b // 8, b % 8
            if t == 1 or kscan_nodeps:
                def prev_lhsT(kc):
                    return hT0[:, kc, :]
            else:
                pb = b - 1
                pm, pi = pb // 8, pb % 8
                def prev_lhsT(kc, pm=pm, pi=pi):
                    return hTm[pm][:, kc, pi * 16:pi * 16 + 16]

            if ps_pending is None:
                ps = psA.tile([NL, H], dt.float32, space="PSUM", tag="scan")
                emit_inject(ps, b)
            else:
                ps = ps_pending
            for hf in range(2):
                for kc in range(KC_H):
                    kc_ = 0 if kscan_same else kc
                    nc.tensor.matmul(
                        out=ps[:, hf * 512:(hf + 1) * 512],
                        lhsT=prev_lhsT(0) if kscan_same else prev_lhsT(kc),
                        rhs=wh_t[:, kc_, hf * 512:(hf + 1) * 512],
                        start=False, stop=(kc == KC_H - 1))
            if t < T_STEPS:
                ps_pending = psA.tile([NL, H], dt.float32, space="PSUM", tag="scan")
                emit_inject(ps_pending, b + 1)
            if kscan_bare:
                continue
            h_scr = scr.tile([NL, H], dt.float32, tag="hscr")
            for hf in range(2):
                nc.scalar.activation(h_scr[:, hf * 512:(hf + 1) * 512],
                                     ps[:, hf * 512:(hf + 1) * 512], AF.Tanh)
            ps_tr = psB.tile([128, KC_H, 16], dt.float32, space="PSUM", tag="tr")
            for kc in range(KC_H):
                nc.tensor.transpose(out=ps_tr[:, kc, :],
                                    in_=h_scr[:, kc * 128:(kc + 1) * 128],
                                    identity=ident16[:])
            nc.vector.tensor_copy(hTm[m][:, :, i * 16:i * 16 + 16], ps_tr[:])

        if kphases == 2 and not kscan_bare:
            nc.sync.dma_start(dbg_ht.ap(), hTm[3][:].bitcast(dt.float32))

        # ---- phase 3: scores + fused CE pieces ----
        with ExitStack() as sctx:
            wvp = sctx.enter_context(tc.tile_pool(name="wvp", bufs=24))
            for j in range(NJ if kphases >= 3 else 0):
                wv_tiles = []
                for kc in range(KC_H):
                    wv_t = wvp.tile([128, 512], dt.float32r, tag="wv")
                    nc.sync.dma_start(wv_t[:], wv_d.ap()[kc, j])
                    wv_tiles.append(wv_t)
                bstage = scr.tile([1, 512], dt.float32r, tag="bstage")
                nc.sync.dma_start(bstage[:], bvoc_d.ap()[j:j + 1, :])
                for m in range(NG):
                    ps = psB.tile([128, 512], dt.float32, space="PSUM", tag="big")
                    for kc in range(KC_H):
                        nc.tensor.matmul(out=ps[:], lhsT=hTm[m][:, kc, :],
                                         rhs=wv_tiles[kc][:],
                                         start=(kc == 0), stop=False)
                    nc.tensor.matmul(out=ps[:], lhsT=ones128[:], rhs=bstage[:],
                                     start=False, stop=True)
                    exp_s = scr.tile([128, 512], dt.float32, tag="exp")
                    nc.scalar.activation(exp_s[:], ps[:], AF.Exp,
                                         accum_out=s_cols[:, m, j:j + 1])
                    stt_s = scr.tile([128, 512], dt.float32, tag="stt")
                    nc.vector.scalar_tensor_tensor(
                        out=stt_s[:], in0=iota_f[:], scalar=yrel_t[:, m, j:j + 1],
                        in1=ps[:], op0=OP.is_equal, op1=OP.mult,
                        accum_out=t_cols[:, m, j:j + 1])

        if kphases == 3:
            nc.sync.dma_start(dbg_sc.ap(), s_cols[:])
            nc.sync.dma_start(dbg_xw.ap()[:, :, :NJ], t_cols[:])
            nc.sync.dma_start(dbg_ht.ap(), hTm[0][:].bitcast(dt.float32))

        # ---- phase 4: loss ----
        if kphases >= 4:
            s_red = acts.tile([128, NG], dt.float32)
            t_red = acts.tile([128, NG], dt.float32)
            for m in range(NG):
                nc.vector.tensor_reduce(out=s_red[:, m:m + 1], in_=s_cols[:, m, :],
                                        axis=mybir.AxisListType.X, op=OP.add)
                nc.vector.tensor_reduce(out=t_red[:, m:m + 1], in_=t_cols[:, m, :],
                                        axis=mybir.AxisListType.X, op=OP.add)
            ln_s = acts.tile([128, NG], dt.float32)
            nc.scalar.activation(ln_s[:], s_red[:], AF.Ln)
            diff = acts.tile([128, NG], dt.float32)
            nc.vector.tensor_tensor(out=diff[:], in0=ln_s[:], in1=t_red[:],
                                    op=OP.subtract)
            masked = acts.tile([128, NG], dt.float32)
            nc.vector.tensor_tensor(out=masked[:], in0=diff[:], in1=maskn_t[:],
                                    op=OP.mult)
            # all-reduce the (128,4) per-partition partial NLLs (2KB payload),
            # then finish the reduction locally on every core
            nc.sync.dma_start(cc_in.ap(), masked[:])
            nc.gpsimd.collective_compute(
                "AllReduce", OP.add,
                replica_groups=[list(range(N_CORES))],
                ins=[cc_in.ap()], outs=[cc_out.ap()])
            red_t = acts.tile([128, NG], dt.float32)
            nc.sync.dma_start(red_t[:], cc_out.ap())
            ps_l = psB.tile([1, NG], dt.float32, space="PSUM", tag="tr")
            nc.tensor.matmul(out=ps_l[:], lhsT=onescol[:], rhs=red_t[:],
                             start=True, stop=True)
            lsb = acts.tile([1, 4], dt.float32)
            nc.vector.tensor_copy(lsb[:], ps_l[:])
            lfin = acts.tile([1, 4], dt.float32)
            nc.vector.memset(lfin[:], 0.0)
            nc.vector.tensor_reduce(out=lfin[:, :1], in_=lsb[:],
                                    axis=mybir.AxisListType.X, op=OP.add)
            nc.sync.dma_start(loss_d.ap(), lfin[:])

    nc.compile()
    _nc_cache = nc
    return nc


def make_in_maps(features, captions, W_proj, b_proj, W_embed, Wx, Wh, b,
                 W_vocab, b_vocab):
    features = np.asarray(features, dtype=np.float32)
    cap = np.asarray(captions).astype(np.int64)
    wv_tiled = np.ascontiguousarray(
        np.asarray(W_vocab, dtype=np.float32)
        .reshape(KC_H, 128, NJ, 512).transpose(0, 2, 1, 3))
    shared = {
        "W_embed": np.asarray(W_embed, dtype=np.float32),
        "W_proj": np.asarray(W_proj, dtype=np.float32),
        "b_proj": np.asarray(b_proj, dtype=np.float32).reshape(1, H),
        "Wx": np.asarray(Wx, dtype=np.float32),
        "Wh": np.asarray(Wh, dtype=np.float32),
        "b_rnn": np.asarray(b, dtype=np.float32).reshape(1, H),
        "WVt": wv_tiled,
        "b_vocab": np.asarray(b_vocab, dtype=np.float32).reshape(NJ, 512),
    }
    in_maps = []
    for c in range(N_CORES):
        capc = cap[c * NL:(c + 1) * NL]              # (16, 33)
        tok_tm = capc[:, :T_STEPS].T.reshape(NTOK)   # token ids, t-major
        y_tm = capc[:, 1:].T.reshape(NTOK)           # targets, t-major
        tok_pg = tok_tm.reshape(NG, 128).T.astype(np.int32).copy()   # (128, NG)
        y_pg = y_tm.reshape(NG, 128).T                               # (128, NG)
        yrel = (y_pg[:, :, None].astype(np.float32)
                - (np.arange(NJ, dtype=np.float32) * 512)[None, None, :])
        maskn = (y_pg != 0).astype(np.float32) / 128.0
        in_maps.append({
            "features": features[c * NL:(c + 1) * NL],
            "tok": tok_pg,
            "yrel": np.ascontiguousarray(yrel),
            "maskn": np.ascontiguousarray(maskn),
            **shared,
        })
    return in_maps


def kernel(**inputs) -> np.ndarray:
    nc = build_program()
    in_maps = make_in_maps(**inputs)
    res = run_bass_kernel_spmd(nc, in_maps, list(range(N_CORES)))
    return np.float32(res.results[0]["loss"][0, 0])

